# revision 15
# baseline (speedup 1.0000x reference)
"""Trainium2 Bass kernel for masked attention (nn_Attention_77704548319841).

Reference computation per batch b:
    CI     = einsum('sc,hc->hs', context[b], W_a)          # [H, S]
    scores = einsum('th,hs->ts', dec[b], CI)               # [T, S]
    scores = where(mask, -1e6, scores)
    attn   = softmax(scores, axis=-1)
    out[b] = einsum('ts,sc->tc', attn, context[b])         # [T, 2H]

Sharding: pure data parallel over batch (16 batches / 8 cores = 2 per core).

v2 design: ZERO device-side transposes.
  - Host supplies every operand pre-transposed + pre-cast to fp16:
      waT  [C, H]      = W_a.T          (mm1 lhsT source)
      ctxT [B, C, S]   = ctx.T          (mm1 rhs)
      ctxN [B, S, C]   = ctx            (mm3 rhs)
      decT [B, H, T]   = dec.T          (mm2 rhs)
      maskT[B, S, T]   = mask.T (u8)
  - Scores are computed TRANSPOSED ([s, t] layout):
      mm1: CI[h, s]     = waT.T @ ctxT          (natural CI layout)
      mm2: scoresT[s,t] = CI.T @ decT           (lhsT = CI, no transpose!)
      softmax over s = over PARTITIONS:
        sm   = maskT*(-1e6) + scoresT           (DVE, f32)
        expT = Exp(sm - SHIFT)  -> bf16         (ACT; bf16 range holds e^±100)
      mm3: out[t,c] = expT.T @ ctxN  (lhsT = expT DIRECTLY - no attn transpose)
        rowsum[t] = expT.T @ ones  (N=1 matmuls sharing mm3's LDWEIGHTS)
        eviction: out_sb = psum * (1/rowsum)  (normalization folded in, fp16)
  - Output stored fp16, host casts back to f32.

Constant-shift softmax is exact w.r.t. the reference (shift cancels; masked
entries exp(s - 1e6 - SHIFT) == 0 identically).

Engine/queue plan:
  gpsimd queue : all big loads, strictly priority-ordered
                 (ctxT b0 h0/h1, decT b0, ctxT b1, ctxN b0, decT b1, maskT b1, ctxN b1)
  sync queue   : waT load (startup), then all 16 output stores
  scalar queue : maskT b0 only (issued at t0 while ACT idle)
  TensorE      : warm-up MMs, then back-to-back matmuls only
  VectorE      : mask+scores fuse (stt), CI evictions, reciprocals, half of
                 mm3 evictions
  ScalarE      : exp, other half of mm3 evictions
"""

import numpy as np
from contextlib import ExitStack

import concourse.bass as bass
import concourse.tile as tile
from concourse import bacc, mybir
from concourse.bass_utils import run_bass_kernel_spmd

B, T, S, H = 16, 1024, 1024, 512
C = 2 * H
N_CORES = 8
BLOC = B // N_CORES  # batches per core
P = 128
NT = T // P   # 8 t-tiles
NS = S // P   # 8 s-tiles
NH = H // P   # 4 h-tiles
NC_ = C // P  # 8 c-tiles
FD = 512      # matmul free-dim chunk
SHIFT = 100.0
NEG_BIG = -1.0e6

f32 = mybir.dt.float32
f16 = mybir.dt.float16
bf16 = mybir.dt.bfloat16
u8 = mybir.dt.uint8
AF = mybir.ActivationFunctionType
ALU = mybir.AluOpType


def _emit(ctx: ExitStack, tc: "tile.TileContext", out_d, decT_d, ctxT_d, ctxN_d,
          mask_d, waT_d):
    nc = tc.nc

    pw = ctx.enter_context(tc.tile_pool(name="pw", bufs=1))
    pin = ctx.enter_context(tc.tile_pool(name="pin", bufs=1))
    ptmp = ctx.enter_context(tc.tile_pool(name="ptmp", bufs=2))
    pout = ctx.enter_context(tc.tile_pool(name="pout", bufs=2))
    pstat = ctx.enter_context(tc.tile_pool(name="pstat", bufs=2))
    ppsum = ctx.enter_context(
        tc.tile_pool(name="ppsum", bufs=2, space=bass.MemorySpace.PSUM)
    )

    # ---- constants ----------------------------------------------------------
    bias_tile = pw.tile([P, 1], f32, tag="bias")
    nc.gpsimd.memset(bias_tile[:], -SHIFT)
    ones_tile = pw.tile([P, 1], bf16, tag="ones")
    nc.gpsimd.memset(ones_tile[:], 1.0)
    wz = pw.tile([P, FD], f16, tag="wz")
    nc.gpsimd.memset(wz[:], 0.0)

    # ---- persistent input tiles --------------------------------------------
    waT = pw.tile([P, NH, NC_, P], f16, tag="waT")      # waT[p, mh, ct, h]
    ctxT = [pin.tile([P, 2, NC_, FD], f16, tag=f"ctxT{b}", name=f"ctxT{b}")
            for b in range(BLOC)]
    ctxN = [pin.tile([P, NS, C], f16, tag=f"ctxN{b}", name=f"ctxN{b}")
            for b in range(BLOC)]
    decT = [pin.tile([P, NH, T], f16, tag=f"decT{b}", name=f"decT{b}")
            for b in range(BLOC)]
    maskT = [pin.tile([P, NS, T], u8, tag=f"maskT{b}", name=f"maskT{b}")
             for b in range(BLOC)]

    # ---- DMA loads (issue order == priority order per queue) ---------------
    # All host-side arrays are exact SBUF tile images: every transfer is
    # fully contiguous per partition (4-16KB lines -> full DMA bandwidth).
    wa_r = waT_d.rearrange("p (mh ct h) -> p mh ct h", mh=NH, ct=NC_)
    # per-mh chunks: mm1 consumes mh in order, each lands just in time
    def load_ctxT(b, half=None, cts=None, eng=None):
        cr = ctxT_d[b].rearrange("p (h ct s) -> p h ct s", h=2, ct=NC_)
        if half is None:
            nc.gpsimd.dma_start(ctxT[b][:], cr)
        elif cts is None:
            nc.gpsimd.dma_start(ctxT[b][:, half], cr[:, half])
        else:
            (eng or nc.gpsimd).dma_start(ctxT[b][:, half, cts[0]:cts[1]],
                                         cr[:, half, cts[0]:cts[1]])

    def load_decT(b):
        nc.gpsimd.dma_start(
            decT[b][:], decT_d[b].rearrange("p (kh t) -> p kh t", kh=NH))

    def load_ctxN(b):
        nc.gpsimd.dma_start(
            ctxN[b][:], ctxN_d[b].rearrange("p (st c) -> p st c", st=NS))

    def load_mask(b):
        nc.gpsimd.dma_start(
            maskT[b][:], mask_d[b].rearrange("p (st t) -> p st t", st=NS))

    # startup: fan the critical bytes over both queues in small chunks so
    # DMA engines ramp fast and mm1's ct-accumulation starts on chunk 0
    nc.sync.dma_start(waT[:, 0, :, :], wa_r[:, 0])     # mh0 first
    load_ctxT(0, 0, cts=(0, 2), eng=nc.sync)
    load_ctxT(0, 0, cts=(2, 4), eng=nc.gpsimd)
    load_ctxT(0, 0, cts=(4, 6), eng=nc.gpsimd)
    load_ctxT(0, 0, cts=(6, 8), eng=nc.gpsimd)
    for mh in range(1, NH):
        nc.sync.dma_start(waT[:, mh, :, :], wa_r[:, mh])
    load_ctxT(0, 1)
    load_decT(0)
    load_mask(0)
    load_ctxT(1)
    load_ctxN(0)
    load_decT(1)
    load_mask(1)
    load_ctxN(1)

    # ---- PE warm-up (HAM) while loads land ---------------------------------
    wps = ppsum.tile([P, FD], f32, tag="psh", bufs=2, name="warm0")
    for _ in range(16):
        nc.tensor.matmul(wps[:], wz[:, 0:P], wz[:], start=True, stop=True)

    # ---- per-batch state ----------------------------------------------------
    CI = [None] * BLOC     # [p, kh, s] fp16 (natural: partitions = h)
    expT = [[None] * NS for _ in range(BLOC)]  # per sc: [p(s), t] bf16
    rr = [None] * BLOC     # [p(t within tc), tc] f32 reciprocal rowsums

    def mm1(b):
        """CI[h, s] = W_a @ ctx[b].T  (accumulate over c)."""
        ci = ptmp.tile([P, NH, S], f16, tag=f"CI{b}", bufs=1, name=f"CI{b}")
        CI[b] = ci
        # b0/ns0 runs during the DMA ramp: ct-outer with 4 open psum groups
        # consumes each arriving ctxT chunk 4 MMs long (matches ramp rate).
        psA = ppsum.tile([P, S], f32, tag="ps", bufs=2, name="psA")
        psB = ppsum.tile([P, S], f32, tag="ps", bufs=2, name="psB")
        grp = [psA[:, 0:FD], psA[:, FD:S], psB[:, 0:FD], psB[:, FD:S]]
        for ct in range(NC_):
            for mh in range(NH):
                nc.tensor.matmul(
                    grp[mh],
                    waT[:, mh, ct, :],
                    ctxT[b][:, 0, ct, :],
                    start=(ct == 0),
                    stop=(ct == NC_ - 1),
                )
        for mh in range(NH):
            nc.vector.tensor_copy(ci[:, mh, 0:FD], grp[mh])
        for mh in range(NH):  # ns=1: mh-outer (data fully loaded by now)
            ps = ppsum.tile([P, FD], f32, tag="psh", bufs=2, name="psh")
            for ct in range(NC_):
                nc.tensor.matmul(
                    ps[:],
                    waT[:, mh, ct, :],
                    ctxT[b][:, 1, ct, :],
                    start=(ct == 0),
                    stop=(ct == NC_ - 1),
                )
            nc.vector.tensor_copy(ci[:, mh, FD:S], ps[:])

    def mm2_softmax(b):
        """scoresT[s, t] per s-chunk; masked exp -> bf16 expT tiles."""
        for sc in range(NS):
            ps = ppsum.tile([P, S], f32, tag="ps", bufs=2, name="ps")
            for kh in range(NH):
                lhs = CI[b][:, kh, sc * P : (sc + 1) * P]
                for th in range(2):
                    nc.tensor.matmul(
                        ps[:, th * FD : (th + 1) * FD],
                        lhs,
                        decT[b][:, kh, th * FD : (th + 1) * FD],
                        start=(kh == 0),
                        stop=(kh == NH - 1),
                    )
            # masked = (maskT * -1e6) + scoresT   (one DVE pass, psum -> sbuf)
            sm = ptmp.tile([P, S], f32, tag="sm", bufs=2, name="sm")
            nc.vector.scalar_tensor_tensor(
                sm[:], maskT[b][:, sc, :], NEG_BIG, ps[:], op0=ALU.mult,
                op1=ALU.add,
            )
            # expT = Exp(masked - SHIFT) in bf16; this IS mm3's lhsT
            ex = ptmp.tile([P, S], bf16, tag=f"expT{sc}", bufs=2,
                           name=f"expT{sc}")
            nc.scalar.activation(ex[:], sm[:], AF.Exp, bias=bias_tile[:],
                                 scale=1.0)
            expT[b][sc] = ex

    def mm3(b):
        """out[t, c] = (expT.T @ ctxN) * (1/rowsum); rowsum via N=1 matmuls."""
        rrt = pstat.tile([P, NT], f32, tag="rr", bufs=2, name="rr")
        rr[b] = rrt
        orr = out_d[b].rearrange("(tt p) c -> p tt c", p=P)
        for mt in range(NT):
            ps = ppsum.tile([P, C], f32, tag="ps", bufs=2, name="ps")
            rs = ppsum.tile([P, 1], f32, tag="rs", bufs=2, name="rs")
            tsl = slice(mt * P, (mt + 1) * P)
            for ks in range(NS):
                lhs = expT[b][ks][:, tsl]
                for nck in range(2):
                    nc.tensor.matmul(
                        ps[:, nck * FD : (nck + 1) * FD],
                        lhs,
                        ctxN[b][:, ks, nck * FD : (nck + 1) * FD],
                        start=(ks == 0),
                        stop=(ks == NS - 1),
                    )
                nc.tensor.matmul(rs[:], lhs, ones_tile[:],
                                 start=(ks == 0), stop=(ks == NS - 1))
            nc.vector.reciprocal(rrt[:, mt : mt + 1], rs[:])
            ob = pout.tile([P, C], f16, tag="ob", bufs=2, name="ob")
            # halves evicted on both engines in parallel, stored on two queues
            nc.scalar.activation(ob[:, 0:FD], ps[:, 0:FD], AF.Copy, bias=0.0,
                                 scale=rrt[:, mt : mt + 1])
            nc.vector.tensor_scalar_mul(ob[:, FD:C], ps[:, FD:C],
                                        rrt[:, mt : mt + 1])
            nc.sync.dma_start(orr[:, mt, 0:FD], ob[:, 0:FD])
            nc.scalar.dma_start(orr[:, mt, FD:C], ob[:, FD:C])

    mm1(0)
    mm2_softmax(0)
    mm1(1)
    mm3(0)
    mm2_softmax(1)
    mm3(1)


_BUILT = None


def _build():
    global _BUILT
    if _BUILT is not None:
        return _BUILT
    nc = bacc.Bacc("TRN2", target_bir_lowering=False, debug=False)
    decT_d = nc.dram_tensor("decT", [BLOC, P, NH * T], f16, kind="ExternalInput")
    ctxT_d = nc.dram_tensor("ctxT", [BLOC, P, C * S // P], f16, kind="ExternalInput")
    ctxN_d = nc.dram_tensor("ctxN", [BLOC, P, S * C // P], f16, kind="ExternalInput")
    mask_d = nc.dram_tensor("maskT", [BLOC, P, S * T // P], u8, kind="ExternalInput")
    waT_d = nc.dram_tensor("waT", [P, C * H // P], f16, kind="ExternalInput")
    out_d = nc.dram_tensor("out", [BLOC, T, C], f16, kind="ExternalOutput")
    with tile.TileContext(nc) as tc, ExitStack() as ctx:
        _emit(ctx, tc, out_d.ap(), decT_d.ap(), ctxT_d.ap(), ctxN_d.ap(),
              mask_d.ap(), waT_d.ap())
    nc.compile()
    _BUILT = nc
    return nc


def make_in_maps(decoder_output, context, mask, W_a):
    dec = np.asarray(decoder_output, dtype=np.float32)
    ctx = np.asarray(context, dtype=np.float32)
    msk = np.asarray(mask)
    wa = np.asarray(W_a, dtype=np.float32)

    # Pack every tensor as the exact SBUF tile image [*, 128, X] so device
    # loads are single fully-contiguous-per-partition transfers.
    # decT tile [p, kh, t] = dec[b, t, kh*128+p]
    decT = np.ascontiguousarray(
        dec.transpose(0, 2, 1).reshape(B, NH, P, T).transpose(0, 2, 1, 3)
        .reshape(B, P, NH * T).astype(np.float16))
    # ctxT tile [p, half, ct, s2] = ctx[b, half*512+s2, ct*128+p]
    ctxT = np.ascontiguousarray(
        ctx.transpose(0, 2, 1).reshape(B, NC_, P, 2, FD)
        .transpose(0, 2, 3, 1, 4).reshape(B, P, C * S // P).astype(np.float16))
    # ctxN tile [p, st, c] = ctx[b, st*128+p, c]
    ctxN = np.ascontiguousarray(
        ctx.reshape(B, NS, P, C).transpose(0, 2, 1, 3)
        .reshape(B, P, S * C // P).astype(np.float16))
    # maskT tile [p, st, t] = mask[b, t, st*128+p]
    maskT = np.ascontiguousarray(
        msk.transpose(0, 2, 1).reshape(B, NS, P, T).transpose(0, 2, 1, 3)
        .reshape(B, P, S * T // P)).astype(np.uint8)
    # waT tile [p, mh, ct, h2] = W_a[mh*128+h2, ct*128+p]
    waT = np.ascontiguousarray(
        wa.T.reshape(NC_, P, NH, P).transpose(1, 2, 0, 3)
        .reshape(P, C * H // P).astype(np.float16))

    in_maps = []
    for i in range(N_CORES):
        sl = slice(i * BLOC, (i + 1) * BLOC)
        in_maps.append(
            {
                "decT": decT[sl],
                "ctxT": ctxT[sl],
                "ctxN": ctxN[sl],
                "maskT": maskT[sl],
                "waT": waT,
            }
        )
    return in_maps


def kernel(decoder_output, context, mask, W_a, **run_kwargs):
    nc = _build()
    in_maps = make_in_maps(decoder_output, context, mask, W_a)
    res = run_bass_kernel_spmd(nc, in_maps, core_ids=list(range(N_CORES)), **run_kwargs)
    out = np.concatenate([res.results[i]["out"] for i in range(N_CORES)], axis=0)
    return out.astype(np.float32)


if __name__ == "__main__":
    nc = _build()
    print("build + compile OK")


# revision 16
# speedup vs baseline: 1.0507x; 1.0507x over previous
"""Trainium2 Bass kernel for masked attention (nn_Attention_77704548319841).

Reference computation per batch b:
    CI     = einsum('sc,hc->hs', context[b], W_a)          # [H, S]
    scores = einsum('th,hs->ts', dec[b], CI)               # [T, S]
    scores = where(mask, -1e6, scores)
    attn   = softmax(scores, axis=-1)
    out[b] = einsum('ts,sc->tc', attn, context[b])         # [T, 2H]

Sharding: pure data parallel over batch (16 batches / 8 cores = 2 per core).

v2 design: ZERO device-side transposes.
  - Host supplies every operand pre-transposed + pre-cast to fp16:
      waT  [C, H]      = W_a.T          (mm1 lhsT source)
      ctxT [B, C, S]   = ctx.T          (mm1 rhs)
      ctxN [B, S, C]   = ctx            (mm3 rhs)
      decT [B, H, T]   = dec.T          (mm2 rhs)
      maskT[B, S, T]   = mask.T (u8)
  - Scores are computed TRANSPOSED ([s, t] layout):
      mm1: CI[h, s]     = waT.T @ ctxT          (natural CI layout)
      mm2: scoresT[s,t] = CI.T @ decT           (lhsT = CI, no transpose!)
      softmax over s = over PARTITIONS:
        sm   = maskT*(-1e6) + scoresT           (DVE, f32)
        expT = Exp(sm - SHIFT)  -> bf16         (ACT; bf16 range holds e^±100)
      mm3: out[t,c] = expT.T @ ctxN  (lhsT = expT DIRECTLY - no attn transpose)
        rowsum[t] = expT.T @ ones  (N=1 matmuls sharing mm3's LDWEIGHTS)
        eviction: out_sb = psum * (1/rowsum)  (normalization folded in, fp16)
  - Output stored fp16, host casts back to f32.

Constant-shift softmax is exact w.r.t. the reference (shift cancels; masked
entries exp(s - 1e6 - SHIFT) == 0 identically).

Engine/queue plan:
  gpsimd queue : all big loads, strictly priority-ordered
                 (ctxT b0 h0/h1, decT b0, ctxT b1, ctxN b0, decT b1, maskT b1, ctxN b1)
  sync queue   : waT load (startup), then all 16 output stores
  scalar queue : maskT b0 only (issued at t0 while ACT idle)
  TensorE      : warm-up MMs, then back-to-back matmuls only
  VectorE      : mask+scores fuse (stt), CI evictions, reciprocals, half of
                 mm3 evictions
  ScalarE      : exp, other half of mm3 evictions
"""

import numpy as np
from contextlib import ExitStack

import concourse.bass as bass
import concourse.tile as tile
from concourse import bacc, mybir
from concourse.bass_utils import run_bass_kernel_spmd

B, T, S, H = 16, 1024, 1024, 512
C = 2 * H
N_CORES = 8
BLOC = B // N_CORES  # batches per core
P = 128
NT = T // P   # 8 t-tiles
NS = S // P   # 8 s-tiles
NH = H // P   # 4 h-tiles
NC_ = C // P  # 8 c-tiles
FD = 512      # matmul free-dim chunk
SHIFT = 100.0
NEG_BIG = -1.0e6

f32 = mybir.dt.float32
f16 = mybir.dt.float16
bf16 = mybir.dt.bfloat16
u8 = mybir.dt.uint8
AF = mybir.ActivationFunctionType
ALU = mybir.AluOpType


def _emit(ctx: ExitStack, tc: "tile.TileContext", out_d, decT_d, ctxT_d, ctxN_d,
          mask_d, waT_d):
    nc = tc.nc

    pw = ctx.enter_context(tc.tile_pool(name="pw", bufs=1))
    pin = ctx.enter_context(tc.tile_pool(name="pin", bufs=1))
    ptmp = ctx.enter_context(tc.tile_pool(name="ptmp", bufs=2))
    pout = ctx.enter_context(tc.tile_pool(name="pout", bufs=2))
    pstat = ctx.enter_context(tc.tile_pool(name="pstat", bufs=2))
    ppsum = ctx.enter_context(
        tc.tile_pool(name="ppsum", bufs=2, space=bass.MemorySpace.PSUM)
    )

    # ---- constants ----------------------------------------------------------
    bias_tile = pw.tile([P, 1], f32, tag="bias")
    nc.gpsimd.memset(bias_tile[:], -SHIFT)
    ones_tile = pw.tile([P, 1], bf16, tag="ones")
    nc.gpsimd.memset(ones_tile[:], 1.0)
    wz = pw.tile([P, FD], f16, tag="wz")
    nc.gpsimd.memset(wz[:], 0.0)

    # ---- persistent input tiles --------------------------------------------
    waT = pw.tile([P, NH, NC_, P], f16, tag="waT")      # waT[p, mh, ct, h]
    ctxT = [pin.tile([P, 2, NC_, FD], f16, tag=f"ctxT{b}", name=f"ctxT{b}")
            for b in range(BLOC)]
    ctxN = [pin.tile([P, NS, C], f16, tag=f"ctxN{b}", name=f"ctxN{b}")
            for b in range(BLOC)]
    decT = [pin.tile([P, NH, T], f16, tag=f"decT{b}", name=f"decT{b}")
            for b in range(BLOC)]
    maskT = [pin.tile([P, NS, T], u8, tag=f"maskT{b}", name=f"maskT{b}")
             for b in range(BLOC)]

    # ---- DMA loads (issue order == priority order per queue) ---------------
    # All host-side arrays are exact SBUF tile images: every transfer is
    # fully contiguous per partition (4-16KB lines -> full DMA bandwidth).
    wa_r = waT_d.rearrange("p (mh ct h) -> p mh ct h", mh=NH, ct=NC_)
    # per-mh chunks: mm1 consumes mh in order, each lands just in time
    def load_ctxT(b, half=None, cts=None, eng=None):
        cr = ctxT_d[b].rearrange("p (h ct s) -> p h ct s", h=2, ct=NC_)
        if half is None:
            nc.gpsimd.dma_start(ctxT[b][:], cr)
        elif cts is None:
            nc.gpsimd.dma_start(ctxT[b][:, half], cr[:, half])
        else:
            (eng or nc.gpsimd).dma_start(ctxT[b][:, half, cts[0]:cts[1]],
                                         cr[:, half, cts[0]:cts[1]])

    def load_decT(b):
        nc.gpsimd.dma_start(
            decT[b][:], decT_d[b].rearrange("p (kh t) -> p kh t", kh=NH))

    def load_ctxN(b):
        nc.gpsimd.dma_start(
            ctxN[b][:], ctxN_d[b].rearrange("p (st c) -> p st c", st=NS))

    def load_mask(b):
        nc.gpsimd.dma_start(
            maskT[b][:], mask_d[b].rearrange("p (st t) -> p st t", st=NS))

    # startup: fan the critical bytes over both queues in small chunks so
    # DMA engines ramp fast and mm1's ct-accumulation starts on chunk 0
    nc.sync.dma_start(waT[:, 0, :, :], wa_r[:, 0])     # mh0 first
    load_ctxT(0, 0, cts=(0, 2), eng=nc.sync)
    load_ctxT(0, 0, cts=(2, 4), eng=nc.gpsimd)
    load_ctxT(0, 0, cts=(4, 6), eng=nc.gpsimd)
    load_ctxT(0, 0, cts=(6, 8), eng=nc.gpsimd)
    for mh in range(1, NH):
        nc.sync.dma_start(waT[:, mh, :, :], wa_r[:, mh])
    load_ctxT(0, 1)
    load_decT(0)
    load_mask(0)
    load_ctxT(1)
    load_ctxN(0)
    load_decT(1)
    load_mask(1)
    load_ctxN(1)

    # ---- PE warm-up (HAM) while loads land ---------------------------------
    wps = ppsum.tile([P, FD], f32, tag="psh", bufs=2, name="warm0")
    for _ in range(10):
        nc.tensor.matmul(wps[:], wz[:, 0:P], wz[:], start=True, stop=True)

    # ---- per-batch state ----------------------------------------------------
    CI = [None] * BLOC     # [p, kh, s] fp16 (natural: partitions = h)
    expT = [[None] * NS for _ in range(BLOC)]  # per sc: [p(s), t] bf16
    rr = [None] * BLOC     # [p(t within tc), tc] f32 reciprocal rowsums

    def mm1(b):
        """CI[h, s] = W_a @ ctx[b].T  (accumulate over c)."""
        ci = ptmp.tile([P, NH, S], f16, tag=f"CI{b}", bufs=1, name=f"CI{b}")
        CI[b] = ci
        for ns in range(2):
            for mh in range(NH):
                ps = ppsum.tile([P, FD], f32, tag="psh", bufs=2, name="psh")
                for ct in range(NC_):
                    nc.tensor.matmul(
                        ps[:],
                        waT[:, mh, ct, :],
                        ctxT[b][:, ns, ct, :],
                        start=(ct == 0),
                        stop=(ct == NC_ - 1),
                    )
                nc.vector.tensor_copy(ci[:, mh, ns * FD : ns * FD + FD], ps[:])

    def mm2_softmax(b):
        """scoresT[s, t] per s-chunk; masked exp -> bf16 expT tiles."""
        for sc in range(NS):
            ps = ppsum.tile([P, S], f32, tag="ps", bufs=2, name="ps")
            for kh in range(NH):
                lhs = CI[b][:, kh, sc * P : (sc + 1) * P]
                for th in range(2):
                    nc.tensor.matmul(
                        ps[:, th * FD : (th + 1) * FD],
                        lhs,
                        decT[b][:, kh, th * FD : (th + 1) * FD],
                        start=(kh == 0),
                        stop=(kh == NH - 1),
                    )
            # masked = (maskT * -1e6) + scoresT   (one DVE pass, psum -> sbuf)
            sm = ptmp.tile([P, S], f32, tag="sm", bufs=2, name="sm")
            nc.vector.scalar_tensor_tensor(
                sm[:], maskT[b][:, sc, :], NEG_BIG, ps[:], op0=ALU.mult,
                op1=ALU.add,
            )
            # expT = Exp(masked - SHIFT) in bf16; this IS mm3's lhsT
            ex = ptmp.tile([P, S], bf16, tag=f"expT{sc}", bufs=2,
                           name=f"expT{sc}")
            nc.scalar.activation(ex[:], sm[:], AF.Exp, bias=bias_tile[:],
                                 scale=1.0)
            expT[b][sc] = ex

    def mm3(b):
        """out[t, c] = (expT.T @ ctxN) * (1/rowsum); rowsum via N=1 matmuls."""
        rrt = pstat.tile([P, NT], f32, tag="rr", bufs=2, name="rr")
        rr[b] = rrt
        orr = out_d[b].rearrange("(tt p) c -> p tt c", p=P)
        for mt in range(NT):
            ps = ppsum.tile([P, C], f32, tag="ps", bufs=2, name="ps")
            rs = ppsum.tile([P, 1], f32, tag="rs", bufs=2, name="rs")
            tsl = slice(mt * P, (mt + 1) * P)
            for ks in range(NS):
                lhs = expT[b][ks][:, tsl]
                for nck in range(2):
                    nc.tensor.matmul(
                        ps[:, nck * FD : (nck + 1) * FD],
                        lhs,
                        ctxN[b][:, ks, nck * FD : (nck + 1) * FD],
                        start=(ks == 0),
                        stop=(ks == NS - 1),
                    )
                nc.tensor.matmul(rs[:], lhs, ones_tile[:],
                                 start=(ks == 0), stop=(ks == NS - 1))
            nc.vector.reciprocal(rrt[:, mt : mt + 1], rs[:])
            ob = pout.tile([P, C], f16, tag="ob", bufs=2, name="ob")
            if mt % 2 == 0:
                nc.scalar.activation(ob[:], ps[:], AF.Copy, bias=0.0,
                                     scale=rrt[:, mt : mt + 1])
            else:
                nc.vector.tensor_scalar_mul(ob[:], ps[:], rrt[:, mt : mt + 1])
            nc.sync.dma_start(orr[:, mt, :], ob[:])

    mm1(0)
    mm2_softmax(0)
    mm1(1)
    mm3(0)
    mm2_softmax(1)
    mm3(1)


_BUILT = None


def _build():
    global _BUILT
    if _BUILT is not None:
        return _BUILT
    nc = bacc.Bacc("TRN2", target_bir_lowering=False, debug=False)
    decT_d = nc.dram_tensor("decT", [BLOC, P, NH * T], f16, kind="ExternalInput")
    ctxT_d = nc.dram_tensor("ctxT", [BLOC, P, C * S // P], f16, kind="ExternalInput")
    ctxN_d = nc.dram_tensor("ctxN", [BLOC, P, S * C // P], f16, kind="ExternalInput")
    mask_d = nc.dram_tensor("maskT", [BLOC, P, S * T // P], u8, kind="ExternalInput")
    waT_d = nc.dram_tensor("waT", [P, C * H // P], f16, kind="ExternalInput")
    out_d = nc.dram_tensor("out", [BLOC, T, C], f16, kind="ExternalOutput")
    with tile.TileContext(nc) as tc, ExitStack() as ctx:
        _emit(ctx, tc, out_d.ap(), decT_d.ap(), ctxT_d.ap(), ctxN_d.ap(),
              mask_d.ap(), waT_d.ap())
    nc.compile()
    _BUILT = nc
    return nc


def make_in_maps(decoder_output, context, mask, W_a):
    dec = np.asarray(decoder_output, dtype=np.float32)
    ctx = np.asarray(context, dtype=np.float32)
    msk = np.asarray(mask)
    wa = np.asarray(W_a, dtype=np.float32)

    # Pack every tensor as the exact SBUF tile image [*, 128, X] so device
    # loads are single fully-contiguous-per-partition transfers.
    # decT tile [p, kh, t] = dec[b, t, kh*128+p]
    decT = np.ascontiguousarray(
        dec.transpose(0, 2, 1).reshape(B, NH, P, T).transpose(0, 2, 1, 3)
        .reshape(B, P, NH * T).astype(np.float16))
    # ctxT tile [p, half, ct, s2] = ctx[b, half*512+s2, ct*128+p]
    ctxT = np.ascontiguousarray(
        ctx.transpose(0, 2, 1).reshape(B, NC_, P, 2, FD)
        .transpose(0, 2, 3, 1, 4).reshape(B, P, C * S // P).astype(np.float16))
    # ctxN tile [p, st, c] = ctx[b, st*128+p, c]
    ctxN = np.ascontiguousarray(
        ctx.reshape(B, NS, P, C).transpose(0, 2, 1, 3)
        .reshape(B, P, S * C // P).astype(np.float16))
    # maskT tile [p, st, t] = mask[b, t, st*128+p]
    maskT = np.ascontiguousarray(
        msk.transpose(0, 2, 1).reshape(B, NS, P, T).transpose(0, 2, 1, 3)
        .reshape(B, P, S * T // P)).astype(np.uint8)
    # waT tile [p, mh, ct, h2] = W_a[mh*128+h2, ct*128+p]
    waT = np.ascontiguousarray(
        wa.T.reshape(NC_, P, NH, P).transpose(1, 2, 0, 3)
        .reshape(P, C * H // P).astype(np.float16))

    in_maps = []
    for i in range(N_CORES):
        sl = slice(i * BLOC, (i + 1) * BLOC)
        in_maps.append(
            {
                "decT": decT[sl],
                "ctxT": ctxT[sl],
                "ctxN": ctxN[sl],
                "maskT": maskT[sl],
                "waT": waT,
            }
        )
    return in_maps


def kernel(decoder_output, context, mask, W_a, **run_kwargs):
    nc = _build()
    in_maps = make_in_maps(decoder_output, context, mask, W_a)
    res = run_bass_kernel_spmd(nc, in_maps, core_ids=list(range(N_CORES)), **run_kwargs)
    out = np.concatenate([res.results[i]["out"] for i in range(N_CORES)], axis=0)
    return out.astype(np.float32)


if __name__ == "__main__":
    nc = _build()
    print("build + compile OK")
